# revision 31
# baseline (speedup 1.0000x reference)
"""Trainium2 Bass kernel for nn_ASS_JRG (gnn_message_passing).

Takes FULL unsharded inputs, shards batch across 8 NeuronCores (pure data
parallel), computes on device via a Bass/Tile kernel, gathers full outputs.

Returns (fused2 (128,512) f32, cell0 (128,10,8,5,512) f32) matching the
reference.
"""
import os
import sys

sys.path.insert(0, '/opt/trn_rl_repo')

import ml_dtypes
import numpy as np

import concourse.bass as bass
import concourse.mybir as mybir
import concourse.tile as tile
from concourse import bacc
from concourse.bass_utils import run_bass_kernel_spmd

F32 = mybir.dt.float32
F32R = mybir.dt.float32r
BF16 = mybir.dt.bfloat16

# Problem dims (hardcoded per spec)
B, T, J, M, D, WHOLE, H = 128, 10, 8, 5, 128, 1024, 512
S = T * J * M            # 400 rows per batch
NCORES = 8
BS = B // NCORES         # 16 batches per core
KC = WHOLE // 128        # 8 contraction chunks
CHUNKS = [(0, 120), (120, 120), (240, 120), (360, 40)]  # 3t,3t,3t,1t row chunks
ABS_KC_DVE = 3           # kc 0:3 abs on DVE, kc 3:8 on ACT (load balance)

_JG = (np.eye(J) + np.eye(J, k=1) + np.eye(J, k=-1)).astype(np.float32)

_cached = {}


def _build_kernel():
    nc = bacc.Bacc("TRN2", target_bir_lowering=False, debug=False,
                   num_devices=NCORES)

    xt_ext = nc.dram_tensor("xt", [BS, 128, KC, S], BF16, kind="ExternalInput").ap()
    ww_ext = nc.dram_tensor("ww", [128, KC, D], BF16, kind="ExternalInput").ap()
    wd_ext = nc.dram_tensor("wd", [128, KC, D], BF16, kind="ExternalInput").ap()
    bw_ext = nc.dram_tensor("bw", [D], F32, kind="ExternalInput").ap()
    bd_ext = nc.dram_tensor("bd", [D], F32, kind="ExternalInput").ap()
    gp_ext = nc.dram_tensor("gpack", [128, 160], BF16, kind="ExternalInput").ap()
    qoh_ext = nc.dram_tensor("qoh", [128, BS, 4, BS], BF16, kind="ExternalInput").ap()
    id_ext = nc.dram_tensor("ident", [128, 128], F32R, kind="ExternalInput").ap()
    idb_ext = nc.dram_tensor("identb", [128, 128], BF16, kind="ExternalInput").ap()
    wa1_ext = nc.dram_tensor("wa1", [128, 4, H], F32R, kind="ExternalInput").ap()
    ba1_ext = nc.dram_tensor("ba1", [128, 4], F32, kind="ExternalInput").ap()

    cell0_ext = nc.dram_tensor("cell0", [BS, S, 4 * D], BF16,
                               kind="ExternalOutput").ap()
    fused2_ext = nc.dram_tensor("fused2", [BS, H], F32,
                                kind="ExternalOutput").ap()

    with tile.TileContext(nc) as tc:
        with tc.tile_pool(name="consts", bufs=1) as cpool, \
             tc.tile_pool(name="persist_ps", bufs=1, space="PSUM") as pps, \
             tc.tile_pool(name="xt", bufs=3) as xpool:
            # ---- prefetch first input batches + matmul weights first ----
            pre_xt = []
            for b in range(2):
                xt = xpool.tile([128, KC, S], BF16, tag="xt")
                nc.sync.dma_start(out=xt[:, :, :], in_=xt_ext[b])
                pre_xt.append(xt)
            # ---- constants ----
            ww_sb = cpool.tile([128, KC, D], BF16)
            nc.sync.dma_start(out=ww_sb[:, :, :], in_=ww_ext[:, :, :])
            wd_sb = cpool.tile([128, KC, D], BF16)
            nc.sync.dma_start(out=wd_sb[:, :, :], in_=wd_ext[:, :, :])
            bw_sb = cpool.tile([128, 1], F32)
            nc.sync.dma_start(out=bw_sb[:, :], in_=bw_ext[:, None])
            bd_sb = cpool.tile([128, 1], F32)
            nc.sync.dma_start(out=bd_sb[:, :], in_=bd_ext[:, None])
            gp_sb = cpool.tile([128, 160], BF16)
            nc.scalar.dma_start(out=gp_sb[:, :], in_=gp_ext[:, :])
            qoh_sb = cpool.tile([128, BS, 4, BS], BF16)
            nc.scalar.dma_start(out=qoh_sb[:, :, :, :], in_=qoh_ext[:, :, :, :])
            id_sb = cpool.tile([128, 128], F32R)
            nc.scalar.dma_start(out=id_sb[:, :], in_=id_ext[:, :])
            idb_sb = cpool.tile([128, 128], BF16)
            nc.scalar.dma_start(out=idb_sb[:, :], in_=idb_ext[:, :])
            wa1_sb = cpool.tile([128, 4, H], F32R)
            nc.scalar.dma_start(out=wa1_sb[:, :, :], in_=wa1_ext[:, :, :])
            ba1_sb = cpool.tile([128, 4], F32)
            nc.scalar.dma_start(out=ba1_sb[:, :], in_=ba1_ext[:, :])

            # per-batch row-sums of relu outputs (for the mean): [:, b, 0]=ew, 1=ed
            mean_sb = cpool.tile([128, BS, 2], F32)
            # qtot accumulation psum, alive across all batches
            qtot_ps = pps.tile([BS, 2 * D], F32)

            n_q = 0  # qtot matmul counter (64 total)
            with tc.tile_pool(name="delta", bufs=2) as dpool, \
                 tc.tile_pool(name="diffr", bufs=2) as fpool, \
                 tc.tile_pool(name="tpose", bufs=3) as tpool, \
                 tc.tile_pool(name="cell", bufs=3) as cellpool, \
                 tc.tile_pool(name="mainps", bufs=3, space="PSUM") as mainps, \
                 tc.tile_pool(name="chunkps", bufs=2, space="PSUM") as chps, \
                 tc.tile_pool(name="pair01", bufs=1, space="PSUM") as chps2:
                for b in range(BS):
                    # ---- load transposed input [128, kc, s] (host-preswizzled,
                    # fully contiguous per partition) ----
                    if b < len(pre_xt):
                        xt = pre_xt[b]
                    else:
                        xt = xpool.tile([128, KC, S], BF16, tag="xt")
                        nc.sync.dma_start(out=xt[:, :, :], in_=xt_ext[b])

                    # ---- temporal diff: |x[s+1] - x[s]|, last col of each
                    # batch-segment 0.  Subtract over the flat [128, 3199]
                    # view (contiguous, fast-mode eligible), then zero the 8
                    # segment-boundary columns before the abs.
                    delta = dpool.tile([128, KC, S], BF16)
                    dflat = delta.rearrange("p kc s -> p (kc s)")
                    xflat = xt.rearrange("p kc s -> p (kc s)")
                    nc.vector.tensor_tensor(
                        out=dflat[:, 0:KC * S - 1], in0=xflat[:, 1:KC * S],
                        in1=xflat[:, 0:KC * S - 1], op=mybir.AluOpType.subtract)
                    nc.vector.memset(delta[:, :, S - 1:S], 0.0)
                    diffr = fpool.tile([128, KC, S], BF16)
                    k0 = ABS_KC_DVE
                    nc.vector.scalar_tensor_tensor(
                        out=diffr[:, 0:k0, :].rearrange("p kc s -> p (kc s)"),
                        in0=delta[:, 0:k0, :].rearrange("p kc s -> p (kc s)"),
                        scalar=-1.0,
                        in1=delta[:, 0:k0, :].rearrange("p kc s -> p (kc s)"),
                        op0=mybir.AluOpType.mult, op1=mybir.AluOpType.max)
                    nc.scalar.activation(
                        out=diffr[:, k0:KC, :], in_=delta[:, k0:KC, :],
                        func=mybir.ActivationFunctionType.Abs)

                    # ---- main matmuls: ewT/edT [128 d, 400 s] ----
                    ew_ps = mainps.tile([128, S], F32, tag="mainps")
                    for kc in range(KC):
                        nc.tensor.matmul(ew_ps[:, :], ww_sb[:, kc, :],
                                         xt[:, kc, :],
                                         start=(kc == 0), stop=(kc == KC - 1))
                    ed_ps = mainps.tile([128, S], F32, tag="mainps")
                    for kc in range(KC):
                        nc.tensor.matmul(ed_ps[:, :], wd_sb[:, kc, :],
                                         diffr[:, kc, :],
                                         start=(kc == 0), stop=(kc == KC - 1))

                    # ---- relu + bias (+ row-sum accumulation) ----
                    ewT = tpool.tile([128, S], BF16, tag="tpose")
                    nc.scalar.activation(out=ewT[:, :], in_=ew_ps[:, :],
                                         func=mybir.ActivationFunctionType.Relu,
                                         bias=bw_sb[:, :], scale=1.0,
                                         accum_out=mean_sb[:, b, 0:1])
                    edT = tpool.tile([128, S], BF16, tag="tpose")
                    nc.scalar.activation(out=edT[:, :], in_=ed_ps[:, :],
                                         func=mybir.ActivationFunctionType.Relu,
                                         bias=bd_sb[:, :], scale=1.0,
                                         accum_out=mean_sb[:, b, 1:2])

                    # ---- chunk phase.  Chunks 0+1 share one 2-bank PSUM
                    # pair tile: their message passing and q-sums each run as
                    # ONE N=512 matmul (same G120 / q weights for both).
                    # Chunks 2,3 run per-chunk.  Un-transposes are emitted
                    # ahead of the matmuls that wait on evacuations. ----
                    cell = cellpool.tile([128, 4, 4 * D], BF16, tag="cell")
                    pair = chps2.tile([128, 2, 4 * D], F32, tag="pair01")
                    for i in (0, 1):
                        c0, cw = CHUNKS[i]
                        nc.tensor.matmul(pair[0:cw, i, 2 * D:3 * D],
                                         ewT[:, c0:c0 + cw], idb_sb[:, :],
                                         start=True, stop=True)
                        nc.tensor.matmul(pair[0:cw, i, 3 * D:4 * D],
                                         edT[:, c0:c0 + cw], idb_sb[:, :],
                                         start=True, stop=True)
                    nc.scalar.activation(
                        out=cell[:, 0:2, 2 * D:4 * D],
                        in_=pair[:, :, 2 * D:4 * D],
                        func=mybir.ActivationFunctionType.Copy)
                    ck_tiles = {}
                    for ch in (2, 3):
                        c0, cw = CHUNKS[ch]
                        ck_ps = chps.tile([128, 4 * D], F32, tag="chunkps")
                        ck_tiles[ch] = ck_ps
                        nc.tensor.matmul(ck_ps[0:cw, 2 * D:3 * D],
                                         ewT[:, c0:c0 + cw], idb_sb[:, :],
                                         start=True, stop=True)
                        nc.tensor.matmul(ck_ps[0:cw, 3 * D:4 * D],
                                         edT[:, c0:c0 + cw], idb_sb[:, :],
                                         start=True, stop=True)
                        nc.scalar.activation(
                            out=cell[0:cw, ch, 2 * D:4 * D],
                            in_=ck_ps[0:cw, 2 * D:4 * D],
                            func=mybir.ActivationFunctionType.Copy)
                    # merged message passing + q-sums for chunks 0+1
                    qtot_bc = bass.AP(
                        tensor=qtot_ps.tensor, offset=qtot_ps.offset,
                        ap=[qtot_ps.ap[0], [0, 2], qtot_ps.ap[1]])
                    nc.tensor.matmul(pair[0:120, :, 0:2 * D],
                                     gp_sb[0:120, 0:120],
                                     cell[0:120, 0:2, 2 * D:4 * D],
                                     start=True, stop=True)
                    nc.tensor.matmul(qtot_bc, qoh_sb[0:120, b, 0, :],
                                     cell[0:120, 0:2, 2 * D:4 * D],
                                     start=(n_q == 0), stop=False,
                                     skip_group_check=True)
                    n_q += 1
                    nc.vector.tensor_copy(cell[:, 0:2, 0:2 * D],
                                          pair[:, :, 0:2 * D])
                    for ch in (2, 3):
                        c0, cw = CHUNKS[ch]
                        ck_ps = ck_tiles[ch]
                        if cw == 120:
                            g_lhsT = gp_sb[0:120, 0:120]
                        else:
                            g_lhsT = gp_sb[0:40, 120:160]
                        nc.tensor.matmul(ck_ps[0:cw, 0:2 * D], g_lhsT,
                                         cell[0:cw, ch, 2 * D:4 * D],
                                         start=True, stop=True)
                        nc.tensor.matmul(qtot_ps[:, :],
                                         qoh_sb[0:cw, b, ch, :],
                                         cell[0:cw, ch, 2 * D:4 * D],
                                         start=False,
                                         stop=(n_q == 3 * BS - 1),
                                         skip_group_check=True)
                        n_q += 1
                        nc.vector.tensor_copy(cell[0:cw, ch, 0:2 * D],
                                              ck_ps[0:cw, 0:2 * D])
                    # ---- store: 2 coalesced DMAs per batch via SWDGE ----
                    nc.gpsimd.dma_start(
                        out=cell0_ext[b, 0:360, :].rearrange(
                            "(ch p) d -> p ch d", p=120),
                        in_=cell[0:120, 0:3, :])
                    nc.gpsimd.dma_start(out=cell0_ext[b, 360:400, :],
                                        in_=cell[0:40, 3, :])

            # ---- tail: fused2 = relu(mean(cell0)) @ W_a1 + b_a1 ----
            with tc.tile_pool(name="tail", bufs=1) as tail, \
                 tc.tile_pool(name="tailps", bufs=2, space="PSUM") as tailps, \
                 tc.tile_pool(name="tailps2", bufs=2, space="PSUM") as tailps2:
                qtot_sb = tail.tile([BS, 2 * D], F32R)
                nc.vector.tensor_copy(qtot_sb[:, :], qtot_ps[:, :])

                fusedT = tail.tile([128, 4, BS], F32R)
                inv_s = 1.0 / S
                # parts 0,1 (ewp/edp means) come from qtot (transposed)
                for part in range(2):
                    tp_ps = tailps.tile([128, BS], F32R, tag="tailps")
                    nc.tensor.transpose(tp_ps[:, :],
                                        qtot_sb[:, part * D:(part + 1) * D],
                                        id_sb[0:BS, 0:BS])
                    nc.scalar.activation(out=fusedT[:, part, :], in_=tp_ps[:, :],
                                         func=mybir.ActivationFunctionType.Relu,
                                         scale=inv_s)
                # parts 2,3 (ew/ed means) from the relu accumulators
                for part, col in ((2, 0), (3, 1)):
                    nc.scalar.activation(out=fusedT[:, part, :],
                                         in_=mean_sb[:, :, col],
                                         func=mybir.ActivationFunctionType.Relu,
                                         scale=inv_s)

                f2nat = tail.tile([BS, H], F32)
                for q in range(4):  # output column chunks of fused2
                    f2_ps = tailps.tile([128, BS], F32, tag="tailps")
                    for hc in range(4):
                        nc.tensor.matmul(f2_ps[:, :],
                                         wa1_sb[:, hc, q * D:(q + 1) * D],
                                         fusedT[:, hc, :],
                                         start=(hc == 0), stop=(hc == 3))
                    f2T = tail.tile([128, BS], F32R, tag="f2T")
                    nc.scalar.activation(out=f2T[:, :], in_=f2_ps[:, :],
                                         func=mybir.ActivationFunctionType.Identity,
                                         bias=ba1_sb[:, q:q + 1], scale=1.0)
                    f2n_ps = tailps2.tile([BS, 128], F32R, tag="tailps2")
                    nc.tensor.transpose(f2n_ps[:, :], f2T[:, :], id_sb[:, :])
                    nc.vector.tensor_copy(f2nat[:, q * D:(q + 1) * D],
                                          f2n_ps[:, :])
                nc.sync.dma_start(out=fused2_ext[:, :], in_=f2nat[:, :])

    nc.compile()
    return nc


def _host_consts(W_whole, b_whole, W_diff, b_diff, general_temporal_mats,
                 W_a1, b_a1):
    g = np.abs(np.asarray(general_temporal_mats)[0] * _JG).astype(np.float32)
    g40 = np.kron(g, np.eye(M, dtype=np.float32))           # (40, 40)
    g120 = np.kron(np.eye(3, dtype=np.float32), g40)        # (120, 120)
    gpack = np.zeros((128, 160), np.float32)
    gpack[0:120, 0:120] = g120
    gpack[0:40, 120:160] = g40

    gsum = g.sum(axis=1)                                    # row sums over k
    qoh = np.zeros((128, BS, 4, BS), np.float32)
    for ch, (c0, cw) in enumerate(CHUNKS):
        s = c0 + np.arange(cw)
        jidx = (s % (J * M)) // M
        qvals = gsum[jidx].astype(np.float32)
        for b in range(BS):
            qoh[0:cw, b, ch, b] = qvals

    bf = ml_dtypes.bfloat16
    ww = np.ascontiguousarray(
        np.asarray(W_whole, np.float32).reshape(KC, 128, D).transpose(1, 0, 2)
    ).astype(bf)
    wd = np.ascontiguousarray(
        np.asarray(W_diff, np.float32).reshape(KC, 128, D).transpose(1, 0, 2)
    ).astype(bf)
    wa1 = np.ascontiguousarray(
        np.asarray(W_a1, np.float32).reshape(4, 128, H).transpose(1, 0, 2))
    ba1 = np.ascontiguousarray(
        np.asarray(b_a1, np.float32).reshape(4, 128).T)

    return {
        "ww": ww,
        "wd": wd,
        "bw": np.ascontiguousarray(b_whole, np.float32),
        "bd": np.ascontiguousarray(b_diff, np.float32),
        "gpack": gpack.astype(bf),
        "qoh": qoh.astype(bf),
        "ident": np.eye(128, dtype=np.float32),
        "identb": np.eye(128, dtype=np.float32).astype(bf),
        "wa1": wa1,
        "ba1": ba1,
    }


def kernel(feat_whole, W_whole, b_whole, W_diff, b_diff,
           general_temporal_mats, W_a1, b_a1):
    feat_whole = np.asarray(feat_whole, np.float32)
    consts = _host_consts(W_whole, b_whole, W_diff, b_diff,
                          general_temporal_mats, W_a1, b_a1)

    if "nc" not in _cached:
        _cached["nc"] = _build_kernel()
    nc = _cached["nc"]

    bf = ml_dtypes.bfloat16
    in_maps = []
    for c in range(NCORES):
        shard = feat_whole[c * BS:(c + 1) * BS]              # (16, 400, 1024)
        # -> [b, p, kc, s]: partition-contiguous transposed layout
        xt = np.ascontiguousarray(
            shard.transpose(0, 2, 1)                          # (16, 1024, 400)
            .reshape(BS, KC, 128, S)
            .transpose(0, 2, 1, 3)).astype(bf)                # (16, 128, 8, 400)
        in_maps.append({"xt": xt, **consts})

    core_ids = list(range(NCORES))
    trace = bool(int(os.environ.get("KERNEL_TRACE", "0")))
    kwargs = {}
    if trace:
        import prof_shim
        prof_shim.install()
        if bool(int(os.environ.get("KERNEL_LDW_OPT", "0"))):
            prof_shim.enable_ldw_opt()
        kwargs = dict(trace=True,
                      tmpdir=os.environ.get("KERNEL_TRACE_DIR", "/tmp/ktrace"))
    res = run_bass_kernel_spmd(nc, in_maps, core_ids, **kwargs)
    _cached["last_exec_time_ns"] = res.exec_time_ns

    cell0 = np.concatenate([res.results[c]["cell0"] for c in range(NCORES)],
                           axis=0)
    fused2 = np.concatenate([res.results[c]["fused2"] for c in range(NCORES)],
                            axis=0)
    cell0 = cell0.astype(np.float32).reshape(B, T, J, M, 4 * D)
    return fused2.astype(np.float32), cell0


# revision 32
# speedup vs baseline: 1.0159x; 1.0159x over previous
"""Trainium2 Bass kernel for nn_ASS_JRG (gnn_message_passing).

Takes FULL unsharded inputs, shards batch across 8 NeuronCores (pure data
parallel), computes on device via a Bass/Tile kernel, gathers full outputs.

Returns (fused2 (128,512) f32, cell0 (128,10,8,5,512) f32) matching the
reference.
"""
import os
import sys

sys.path.insert(0, '/opt/trn_rl_repo')

import ml_dtypes
import numpy as np

import concourse.bass as bass
import concourse.mybir as mybir
import concourse.tile as tile
from concourse import bacc
from concourse.bass_utils import run_bass_kernel_spmd

F32 = mybir.dt.float32
F32R = mybir.dt.float32r
BF16 = mybir.dt.bfloat16

# Problem dims (hardcoded per spec)
B, T, J, M, D, WHOLE, H = 128, 10, 8, 5, 128, 1024, 512
S = T * J * M            # 400 rows per batch
NCORES = 8
BS = B // NCORES         # 16 batches per core
KC = WHOLE // 128        # 8 contraction chunks
CHUNKS = [(0, 120), (120, 120), (240, 120), (360, 40)]  # 3t,3t,3t,1t row chunks
ABS_KC_DVE = 3           # kc 0:3 abs on DVE, kc 3:8 on ACT (load balance)

_JG = (np.eye(J) + np.eye(J, k=1) + np.eye(J, k=-1)).astype(np.float32)

_cached = {}


def _build_kernel():
    nc = bacc.Bacc("TRN2", target_bir_lowering=False, debug=False,
                   num_devices=NCORES)

    xt_ext = nc.dram_tensor("xt", [BS, 128, KC, S], BF16, kind="ExternalInput").ap()
    ww_ext = nc.dram_tensor("ww", [128, KC, D], BF16, kind="ExternalInput").ap()
    wd_ext = nc.dram_tensor("wd", [128, KC, D], BF16, kind="ExternalInput").ap()
    bw_ext = nc.dram_tensor("bw", [D], F32, kind="ExternalInput").ap()
    bd_ext = nc.dram_tensor("bd", [D], F32, kind="ExternalInput").ap()
    gp_ext = nc.dram_tensor("gpack", [128, 160], BF16, kind="ExternalInput").ap()
    qoh_ext = nc.dram_tensor("qoh", [128, BS, 4, BS], BF16, kind="ExternalInput").ap()
    id_ext = nc.dram_tensor("ident", [128, 128], F32R, kind="ExternalInput").ap()
    idb_ext = nc.dram_tensor("identb", [128, 128], BF16, kind="ExternalInput").ap()
    wa1_ext = nc.dram_tensor("wa1", [128, 4, H], F32R, kind="ExternalInput").ap()
    ba1_ext = nc.dram_tensor("ba1", [128, 4], F32, kind="ExternalInput").ap()

    cell0_ext = nc.dram_tensor("cell0", [BS, S, 4 * D], BF16,
                               kind="ExternalOutput").ap()
    fused2_ext = nc.dram_tensor("fused2", [BS, H], F32,
                                kind="ExternalOutput").ap()

    with tile.TileContext(nc) as tc:
        with tc.tile_pool(name="consts", bufs=1) as cpool, \
             tc.tile_pool(name="persist_ps", bufs=1, space="PSUM") as pps, \
             tc.tile_pool(name="xt", bufs=3) as xpool:
            # ---- prefetch first input batches + matmul weights first ----
            pre_xt = []
            for b in range(2):
                xt = xpool.tile([128, KC, S], BF16, tag="xt")
                nc.sync.dma_start(out=xt[:, :, :], in_=xt_ext[b])
                pre_xt.append(xt)
            # ---- constants ----
            ww_sb = cpool.tile([128, KC, D], BF16)
            nc.sync.dma_start(out=ww_sb[:, :, :], in_=ww_ext[:, :, :])
            wd_sb = cpool.tile([128, KC, D], BF16)
            nc.sync.dma_start(out=wd_sb[:, :, :], in_=wd_ext[:, :, :])
            bw_sb = cpool.tile([128, 1], F32)
            nc.sync.dma_start(out=bw_sb[:, :], in_=bw_ext[:, None])
            bd_sb = cpool.tile([128, 1], F32)
            nc.sync.dma_start(out=bd_sb[:, :], in_=bd_ext[:, None])
            gp_sb = cpool.tile([128, 160], BF16)
            nc.scalar.dma_start(out=gp_sb[:, :], in_=gp_ext[:, :])
            qoh_sb = cpool.tile([128, BS, 4, BS], BF16)
            nc.scalar.dma_start(out=qoh_sb[:, :, :, :], in_=qoh_ext[:, :, :, :])
            id_sb = cpool.tile([128, 128], F32R)
            nc.scalar.dma_start(out=id_sb[:, :], in_=id_ext[:, :])
            idb_sb = cpool.tile([128, 128], BF16)
            nc.scalar.dma_start(out=idb_sb[:, :], in_=idb_ext[:, :])
            wa1_sb = cpool.tile([128, 4, H], F32R)
            nc.scalar.dma_start(out=wa1_sb[:, :, :], in_=wa1_ext[:, :, :])
            ba1_sb = cpool.tile([128, 4], F32)
            nc.scalar.dma_start(out=ba1_sb[:, :], in_=ba1_ext[:, :])

            # per-batch row-sums of relu outputs (for the mean): [:, b, 0]=ew, 1=ed
            mean_sb = cpool.tile([128, BS, 2], F32)
            # qtot accumulation psum, alive across all batches
            qtot_ps = pps.tile([BS, 2 * D], F32)

            n_q = 0  # qtot matmul counter (64 total)
            with tc.tile_pool(name="delta", bufs=2) as dpool, \
                 tc.tile_pool(name="diffr", bufs=2) as fpool, \
                 tc.tile_pool(name="tpose", bufs=3) as tpool, \
                 tc.tile_pool(name="cell", bufs=3) as cellpool, \
                 tc.tile_pool(name="mainps", bufs=3, space="PSUM") as mainps, \
                 tc.tile_pool(name="chunkps", bufs=4, space="PSUM") as chps:
                for b in range(BS):
                    # ---- load transposed input [128, kc, s] (host-preswizzled,
                    # fully contiguous per partition) ----
                    if b < len(pre_xt):
                        xt = pre_xt[b]
                    else:
                        xt = xpool.tile([128, KC, S], BF16, tag="xt")
                        nc.sync.dma_start(out=xt[:, :, :], in_=xt_ext[b])

                    # ---- temporal diff: |x[s+1] - x[s]|, last col of each
                    # batch-segment 0.  Subtract over the flat [128, 3199]
                    # view (contiguous, fast-mode eligible), then zero the 8
                    # segment-boundary columns before the abs.
                    delta = dpool.tile([128, KC, S], BF16)
                    dflat = delta.rearrange("p kc s -> p (kc s)")
                    xflat = xt.rearrange("p kc s -> p (kc s)")
                    nc.vector.tensor_tensor(
                        out=dflat[:, 0:KC * S - 1], in0=xflat[:, 1:KC * S],
                        in1=xflat[:, 0:KC * S - 1], op=mybir.AluOpType.subtract)
                    nc.vector.memset(delta[:, :, S - 1:S], 0.0)
                    diffr = fpool.tile([128, KC, S], BF16)
                    k0 = ABS_KC_DVE
                    nc.vector.scalar_tensor_tensor(
                        out=diffr[:, 0:k0, :].rearrange("p kc s -> p (kc s)"),
                        in0=delta[:, 0:k0, :].rearrange("p kc s -> p (kc s)"),
                        scalar=-1.0,
                        in1=delta[:, 0:k0, :].rearrange("p kc s -> p (kc s)"),
                        op0=mybir.AluOpType.mult, op1=mybir.AluOpType.max)
                    nc.scalar.activation(
                        out=diffr[:, k0:KC, :], in_=delta[:, k0:KC, :],
                        func=mybir.ActivationFunctionType.Abs)

                    # ---- main matmuls: ewT/edT [128 d, 400 s] ----
                    ew_ps = mainps.tile([128, S], F32, tag="mainps")
                    for kc in range(KC):
                        nc.tensor.matmul(ew_ps[:, :], ww_sb[:, kc, :],
                                         xt[:, kc, :],
                                         start=(kc == 0), stop=(kc == KC - 1))
                    ed_ps = mainps.tile([128, S], F32, tag="mainps")
                    for kc in range(KC):
                        nc.tensor.matmul(ed_ps[:, :], wd_sb[:, kc, :],
                                         diffr[:, kc, :],
                                         start=(kc == 0), stop=(kc == KC - 1))

                    # ---- relu + bias (+ row-sum accumulation) ----
                    ewT = tpool.tile([128, S], BF16, tag="tpose")
                    nc.scalar.activation(out=ewT[:, :], in_=ew_ps[:, :],
                                         func=mybir.ActivationFunctionType.Relu,
                                         bias=bw_sb[:, :], scale=1.0,
                                         accum_out=mean_sb[:, b, 0:1])
                    edT = tpool.tile([128, S], BF16, tag="tpose")
                    nc.scalar.activation(out=edT[:, :], in_=ed_ps[:, :],
                                         func=mybir.ActivationFunctionType.Relu,
                                         bias=bd_sb[:, :], scale=1.0,
                                         accum_out=mean_sb[:, b, 1:2])

                    # ---- chunk phase, software-pipelined by one chunk:
                    # un-transposes of chunk ch+1 run while chunk ch's
                    # evacuation completes, so MP/qtot never head-block PE ----
                    cell = cellpool.tile([128, 4, 4 * D], BF16, tag="cell")
                    ck_tiles = [None] * 4

                    def emit_ids(ch):
                        c0, cw = CHUNKS[ch]
                        ck_ps = chps.tile([128, 4 * D], F32, tag="chunkps")
                        ck_tiles[ch] = ck_ps
                        nc.tensor.matmul(ck_ps[0:cw, 2 * D:3 * D],
                                         ewT[:, c0:c0 + cw], idb_sb[:, :],
                                         start=True, stop=True)
                        nc.tensor.matmul(ck_ps[0:cw, 3 * D:4 * D],
                                         edT[:, c0:c0 + cw], idb_sb[:, :],
                                         start=True, stop=True)
                        nc.scalar.activation(
                            out=cell[0:cw, ch, 2 * D:4 * D],
                            in_=ck_ps[0:cw, 2 * D:4 * D],
                            func=mybir.ActivationFunctionType.Copy)

                    emit_ids(0)
                    qtot_bc = bass.AP(
                        tensor=qtot_ps.tensor, offset=qtot_ps.offset,
                        ap=[qtot_ps.ap[0], [0, 2], qtot_ps.ap[1]])
                    for ch, (c0, cw) in enumerate(CHUNKS):
                        if ch + 1 < 4:
                            emit_ids(ch + 1)
                        ck_ps = ck_tiles[ch]
                        # message passing over joints: one matmul for ew|ed
                        if cw == 120:
                            g_lhsT = gp_sb[0:120, 0:120]
                        else:
                            g_lhsT = gp_sb[0:40, 120:160]
                        nc.tensor.matmul(ck_ps[0:cw, 0:2 * D], g_lhsT,
                                         cell[0:cw, ch, 2 * D:4 * D],
                                         start=True, stop=True)
                        # graph-weighted row sums for the fused mean.  Chunks
                        # 0+1 share the same q-weight matrix, so one N=512
                        # matmul covers both via a stride-0 output AP that
                        # revisits the same PSUM columns (per-element
                        # has_written accumulates within the matmul).
                        if ch == 0:
                            nc.tensor.matmul(qtot_bc,
                                             qoh_sb[0:cw, b, 0, :],
                                             cell[0:cw, 0:2, 2 * D:4 * D],
                                             start=(n_q == 0), stop=False,
                                             skip_group_check=True)
                            n_q += 1
                        elif ch >= 2:
                            nc.tensor.matmul(qtot_ps[:, :],
                                             qoh_sb[0:cw, b, ch, :],
                                             cell[0:cw, ch, 2 * D:4 * D],
                                             start=False,
                                             stop=(n_q == 3 * BS - 1),
                                             skip_group_check=True)
                            n_q += 1
                        nc.vector.tensor_copy(cell[0:cw, ch, 0:2 * D],
                                              ck_ps[0:cw, 0:2 * D])

                    # ---- store: 2 coalesced DMAs per batch via SWDGE ----
                    nc.gpsimd.dma_start(
                        out=cell0_ext[b, 0:360, :].rearrange(
                            "(ch p) d -> p ch d", p=120),
                        in_=cell[0:120, 0:3, :])
                    nc.gpsimd.dma_start(out=cell0_ext[b, 360:400, :],
                                        in_=cell[0:40, 3, :])

            # ---- tail: fused2 = relu(mean(cell0)) @ W_a1 + b_a1 ----
            with tc.tile_pool(name="tail", bufs=1) as tail, \
                 tc.tile_pool(name="tailps", bufs=2, space="PSUM") as tailps, \
                 tc.tile_pool(name="tailps2", bufs=2, space="PSUM") as tailps2:
                qtot_sb = tail.tile([BS, 2 * D], F32R)
                nc.vector.tensor_copy(qtot_sb[:, :], qtot_ps[:, :])

                fusedT = tail.tile([128, 4, BS], F32R)
                inv_s = 1.0 / S
                # parts 0,1 (ewp/edp means) come from qtot (transposed)
                for part in range(2):
                    tp_ps = tailps.tile([128, BS], F32R, tag="tailps")
                    nc.tensor.transpose(tp_ps[:, :],
                                        qtot_sb[:, part * D:(part + 1) * D],
                                        id_sb[0:BS, 0:BS])
                    nc.scalar.activation(out=fusedT[:, part, :], in_=tp_ps[:, :],
                                         func=mybir.ActivationFunctionType.Relu,
                                         scale=inv_s)
                # parts 2,3 (ew/ed means) from the relu accumulators
                for part, col in ((2, 0), (3, 1)):
                    nc.scalar.activation(out=fusedT[:, part, :],
                                         in_=mean_sb[:, :, col],
                                         func=mybir.ActivationFunctionType.Relu,
                                         scale=inv_s)

                f2nat = tail.tile([BS, H], F32)
                for q in range(4):  # output column chunks of fused2
                    f2_ps = tailps.tile([128, BS], F32, tag="tailps")
                    for hc in range(4):
                        nc.tensor.matmul(f2_ps[:, :],
                                         wa1_sb[:, hc, q * D:(q + 1) * D],
                                         fusedT[:, hc, :],
                                         start=(hc == 0), stop=(hc == 3))
                    f2T = tail.tile([128, BS], F32R, tag="f2T")
                    nc.scalar.activation(out=f2T[:, :], in_=f2_ps[:, :],
                                         func=mybir.ActivationFunctionType.Identity,
                                         bias=ba1_sb[:, q:q + 1], scale=1.0)
                    f2n_ps = tailps2.tile([BS, 128], F32R, tag="tailps2")
                    nc.tensor.transpose(f2n_ps[:, :], f2T[:, :], id_sb[:, :])
                    nc.vector.tensor_copy(f2nat[:, q * D:(q + 1) * D],
                                          f2n_ps[:, :])
                nc.sync.dma_start(out=fused2_ext[:, :], in_=f2nat[:, :])

    nc.compile()
    return nc


def _host_consts(W_whole, b_whole, W_diff, b_diff, general_temporal_mats,
                 W_a1, b_a1):
    g = np.abs(np.asarray(general_temporal_mats)[0] * _JG).astype(np.float32)
    g40 = np.kron(g, np.eye(M, dtype=np.float32))           # (40, 40)
    g120 = np.kron(np.eye(3, dtype=np.float32), g40)        # (120, 120)
    gpack = np.zeros((128, 160), np.float32)
    gpack[0:120, 0:120] = g120
    gpack[0:40, 120:160] = g40

    gsum = g.sum(axis=1)                                    # row sums over k
    qoh = np.zeros((128, BS, 4, BS), np.float32)
    for ch, (c0, cw) in enumerate(CHUNKS):
        s = c0 + np.arange(cw)
        jidx = (s % (J * M)) // M
        qvals = gsum[jidx].astype(np.float32)
        for b in range(BS):
            qoh[0:cw, b, ch, b] = qvals

    bf = ml_dtypes.bfloat16
    ww = np.ascontiguousarray(
        np.asarray(W_whole, np.float32).reshape(KC, 128, D).transpose(1, 0, 2)
    ).astype(bf)
    wd = np.ascontiguousarray(
        np.asarray(W_diff, np.float32).reshape(KC, 128, D).transpose(1, 0, 2)
    ).astype(bf)
    wa1 = np.ascontiguousarray(
        np.asarray(W_a1, np.float32).reshape(4, 128, H).transpose(1, 0, 2))
    ba1 = np.ascontiguousarray(
        np.asarray(b_a1, np.float32).reshape(4, 128).T)

    return {
        "ww": ww,
        "wd": wd,
        "bw": np.ascontiguousarray(b_whole, np.float32),
        "bd": np.ascontiguousarray(b_diff, np.float32),
        "gpack": gpack.astype(bf),
        "qoh": qoh.astype(bf),
        "ident": np.eye(128, dtype=np.float32),
        "identb": np.eye(128, dtype=np.float32).astype(bf),
        "wa1": wa1,
        "ba1": ba1,
    }


def kernel(feat_whole, W_whole, b_whole, W_diff, b_diff,
           general_temporal_mats, W_a1, b_a1):
    feat_whole = np.asarray(feat_whole, np.float32)
    consts = _host_consts(W_whole, b_whole, W_diff, b_diff,
                          general_temporal_mats, W_a1, b_a1)

    if "nc" not in _cached:
        _cached["nc"] = _build_kernel()
    nc = _cached["nc"]

    bf = ml_dtypes.bfloat16
    in_maps = []
    for c in range(NCORES):
        shard = feat_whole[c * BS:(c + 1) * BS]              # (16, 400, 1024)
        # -> [b, p, kc, s]: partition-contiguous transposed layout
        xt = np.ascontiguousarray(
            shard.transpose(0, 2, 1)                          # (16, 1024, 400)
            .reshape(BS, KC, 128, S)
            .transpose(0, 2, 1, 3)).astype(bf)                # (16, 128, 8, 400)
        in_maps.append({"xt": xt, **consts})

    core_ids = list(range(NCORES))
    trace = bool(int(os.environ.get("KERNEL_TRACE", "0")))
    kwargs = {}
    if trace:
        import prof_shim
        prof_shim.install()
        if bool(int(os.environ.get("KERNEL_LDW_OPT", "0"))):
            prof_shim.enable_ldw_opt()
        kwargs = dict(trace=True,
                      tmpdir=os.environ.get("KERNEL_TRACE_DIR", "/tmp/ktrace"))
    res = run_bass_kernel_spmd(nc, in_maps, core_ids, **kwargs)
    _cached["last_exec_time_ns"] = res.exec_time_ns

    cell0 = np.concatenate([res.results[c]["cell0"] for c in range(NCORES)],
                           axis=0)
    fused2 = np.concatenate([res.results[c]["fused2"] for c in range(NCORES)],
                            axis=0)
    cell0 = cell0.astype(np.float32).reshape(B, T, J, M, 4 * D)
    return fused2.astype(np.float32), cell0
